# revision 1
# baseline (speedup 1.0000x reference)
"""Multi-head attention (B=2, N=4096, C=512, H=8, D=64) on 8 TRN2 NeuronCores.

Sharding: data-parallel over batch (2 groups of 4 cores) x tensor-parallel over
heads (2 heads/core). Per core: qkv projection, attention for its 2 heads, and
a partial output projection y_partial^T = Wp_slice^T @ attn^T; the host sums
the 4 per-batch partials, transposes, adds bias.

Engine layout learned from profiling:
- All matmuls bf16. A bf16 moving operand with only 64 partitions streams at
  2 cyc/col, so the per-head QK matmuls use zero-padded full-128-partition
  K tiles (kt0/kt1) against the full 128-partition Q tile.
- exp(S) runs on ScalarE out of 2-bank (128,1024) PSUM tiles; ScalarE and
  the PE are both near-saturated and pace the kernel together.
- Softmax denominator rides the PV matmul as a ones-column appended to V
  (lhsT is [V_h | 1], M=65); the denominator row is moved to partition 0 by a
  small DMA, inverted on VectorE, and partition-broadcast on GpSimd.
"""
import os
import sys

for _p in ("/opt/trn_rl_repo", "/root/.axon_site/_ro/trn_rl_repo"):
    if os.path.isdir(_p) and _p not in sys.path:
        sys.path.append(_p)

import numpy as np
from contextlib import ExitStack

import concourse.bass as bass
import concourse.mybir as mybir
import concourse.tile as tile
from concourse import bacc
from concourse.bass_utils import run_bass_kernel_spmd

F32 = mybir.dt.float32
BF16 = mybir.dt.bfloat16
EXP = mybir.ActivationFunctionType.Exp

DIM, N, HD = 512, 4096, 64
SCALE = HD ** -0.5
NB = N // 512    # 8  n-blocks of 512 queries
MB = N // 128    # 32 m-chunks of 128 keys
CC = DIM // 128  # 4  c-chunks of the model dim
# m-chunks per (nb, h) are processed in 2-bank PSUM tiles of 2 chunks each
TILES = [(t * 2, 2) for t in range(MB // 2)]


def build_nc():
    nc = bacc.Bacc("TRN2", target_bir_lowering=False)
    xT = nc.declare_dram_parameter("xT", [DIM, N], F32, isOutput=False)
    wqkvT = nc.declare_dram_parameter("wqkvT", [DIM, 384], F32, isOutput=False)
    wpT = nc.declare_dram_parameter("wpT", [128, DIM], F32, isOutput=False)
    out = nc.declare_dram_parameter("out", [DIM, N], F32, isOutput=True)

    with ExitStack() as ctx:
        tc = ctx.enter_context(tile.TileContext(nc))
        big = ctx.enter_context(tc.tile_pool(name="big", bufs=1))
        stage = ctx.enter_context(tc.tile_pool(name="stage", bufs=4))
        esp = ctx.enter_context(tc.tile_pool(name="esp", bufs=6))
        yup = ctx.enter_context(tc.tile_pool(name="yup", bufs=2))
        ysp = ctx.enter_context(tc.tile_pool(name="ysp", bufs=3))
        ps_p = ctx.enter_context(tc.tile_pool(name="psA", bufs=3, space="PSUM"))
        po_p = ctx.enter_context(tc.tile_pool(name="psB", bufs=2, space="PSUM"))

        # ---- loads (staged through a recycled f32 pool, cast to bf16 on-chip) ----
        wq = []
        for cc in range(CC):
            t = stage.tile([128, 2048], F32, tag="stage", name="stage")
            nc.sync.dma_start(out=t[:, 0:384], in_=wqkvT[cc * 128:(cc + 1) * 128, :])
            tb = big.tile([128, 384], BF16, tag=f"wqb{cc}", name=f"wqb{cc}")
            nc.gpsimd.tensor_copy(out=tb[:], in_=t[:, 0:384])
            wq.append(tb)
        xtb = []
        for cc in range(CC):
            tb = big.tile([128, N], BF16, tag=f"xtb{cc}", name=f"xtb{cc}")
            xtb.append(tb)
        for half in range(2):
            hs = slice(half * 2048, (half + 1) * 2048)
            for cc in range(CC):
                t = stage.tile([128, 2048], F32, tag="stage", name="stage")
                nc.sync.dma_start(out=t[:], in_=xT[cc * 128:(cc + 1) * 128, hs])
                nc.vector.tensor_copy(out=xtb[cc][:, hs], in_=t[:])
        t = stage.tile([128, 2048], F32, tag="stage", name="stage")
        nc.sync.dma_start(out=t[:, 0:DIM], in_=wpT[:, :])
        wpb = big.tile([128, DIM], BF16, tag="wpb", name="wpb")
        nc.gpsimd.tensor_copy(out=wpb[:], in_=t[:, 0:DIM])

        # V2 layout per m-chunk: [V_h0(64) | 1 | V_h1(64) | 1]
        v2 = big.tile([128, 130 * MB], BF16, tag="v2", name="v2")
        nc.vector.memset(v2[:], 1.0)
        qt = big.tile([128, N], BF16, tag="qt", name="qt")
        # per-head K with the other head's partitions zeroed (full-128-partition
        # moving/stationary operands keep the bf16 stream at 1 cyc/col)
        kt = [big.tile([128, N], BF16, tag=f"kt{h}", name=f"kt{h}") for h in range(2)]
        nc.vector.memset(kt[0][64:128, :], 0.0)
        nc.vector.memset(kt[1][0:64, :], 0.0)
        atB = big.tile([128, N], BF16, tag="atB", name="atB")

        # ---- Q^T / K^T: (qkv chan, n) = W^T . x^T ----
        for blk in range(2):
            for nb in range(NB):
                ps = ps_p.tile([128, 1024], F32, tag="ps", name="ps")[:, 0:512]
                for cc in range(CC):
                    nc.tensor.matmul(
                        ps,
                        lhsT=wq[cc][:, blk * 128:(blk + 1) * 128],
                        rhs=xtb[cc][:, nb * 512:(nb + 1) * 512],
                        start=(cc == 0),
                        stop=(cc == CC - 1),
                    )
                ns = slice(nb * 512, (nb + 1) * 512)
                if blk == 0:
                    nc.vector.tensor_copy(out=qt[:, ns], in_=ps)
                else:
                    nc.vector.tensor_copy(out=kt[0][0:64, ns], in_=ps[0:64, :])
                    nc.vector.tensor_copy(out=kt[1][64:128, ns], in_=ps[64:128, :])

        # ---- V, m-partitioned: V[m, vchan] = x[m,:] . Wv^T ----
        for mb in range(MB):
            ps = ps_p.tile([128, 1024], F32, tag="ps", name="ps")[:, 0:512]
            for cc in range(CC):
                nc.tensor.matmul(
                    ps[:, 0:128],
                    lhsT=xtb[cc][:, mb * 128:(mb + 1) * 128],
                    rhs=wq[cc][:, 256:384],
                    start=(cc == 0),
                    stop=(cc == CC - 1),
                )
            nc.vector.tensor_copy(out=v2[:, mb * 130:mb * 130 + 64], in_=ps[:, 0:64])
            nc.vector.tensor_copy(out=v2[:, mb * 130 + 65:mb * 130 + 129], in_=ps[:, 64:128])

        # ---- attention + partial proj per n-block ----
        # proj(nb-1) is emitted between the h0 and h1 groups of nb: by then its
        # input chain is long done, so those matmuls fill the pipeline drain at
        # the group boundary instead of stalling on the last exp.
        def emit_proj(nb):
            ns = slice(nb * 512, (nb + 1) * 512)
            for ob in range(4):
                pp = po_p.tile([128, 512], F32, tag="po", name="pp")
                nc.tensor.matmul(
                    pp[:],
                    lhsT=wpb[:, ob * 128:(ob + 1) * 128],
                    rhs=atB[:, ns],
                    start=True,
                    stop=True,
                )
                ys = ysp.tile([128, 512], F32, tag="ys", name="ys")
                nc.vector.tensor_copy(out=ys[:], in_=pp[:])
                nc.sync.dma_start(out=out[ob * 128:(ob + 1) * 128, ns], in_=ys[:])

        def emit_norm(nb, h, po):
            ns = slice(nb * 512, (nb + 1) * 512)
            yu = yup.tile([128, 512], F32, tag="yu", name="yu")
            nc.vector.tensor_copy(out=yu[0:65, :], in_=po[0:65, :])
            row = yup.tile([1, 512], F32, tag="row", name="row")
            nc.sync.dma_start(out=row[:], in_=yu[64:65, :])
            den = yup.tile([64, 512], F32, tag="den", name="den")
            nc.gpsimd.partition_broadcast(den[:], row[0:1, :])
            rec = yup.tile([64, 512], F32, tag="rec", name="rec")
            nc.vector.reciprocal_approx_fast(out=rec[:], in_=den[:])
            if h == 0:
                nc.vector.tensor_mul(out=atB[0:64, ns], in0=yu[0:64, :], in1=rec[:])
                if nb > 0:
                    emit_proj(nb - 1)
            else:
                a1 = yup.tile([64, 512], BF16, tag="a1", name="a1")
                nc.vector.tensor_mul(out=a1[:], in0=yu[0:64, :], in1=rec[:])
                nc.sync.dma_start(out=atB[64:128, ns], in_=a1[:])

        pend = None  # (nb, h, po, es_of_last_tile)
        for nb in range(NB):
            ns = slice(nb * 512, (nb + 1) * 512)
            for h in range(2):
                po = po_p.tile([128, 512], F32, tag="po", name="po")
                esL = []
                for t, (mb0, w) in enumerate(TILES):
                    ps = ps_p.tile([128, 1024], F32, tag="ps", name="ps")
                    for j in range(w):
                        mb = mb0 + j
                        nc.tensor.matmul(
                            ps[:, j * 512:(j + 1) * 512],
                            lhsT=kt[h][:, mb * 128:(mb + 1) * 128],
                            rhs=qt[:, ns],
                            start=True,
                            stop=True,
                        )
                    es = esp.tile([128, 1024], BF16, tag="es", name="es")
                    nc.scalar.activation(
                        out=es[:, 0:w * 512], in_=ps[:, 0:w * 512], func=EXP, scale=SCALE
                    )
                    esL.append(es)
                    if t == 0 and pend is not None:
                        # flush the previous group: its last PV pair + norm run
                        # while this group's first exp is still in flight
                        pnb, ph, ppo, pes = pend
                        pmb0, pw = TILES[-1]
                        for j in range(pw):
                            mb = pmb0 + j
                            nc.tensor.matmul(
                                ppo[0:65, :],
                                lhsT=v2[:, mb * 130 + 65 * ph:mb * 130 + 65 * ph + 65],
                                rhs=pes[:, j * 512:(j + 1) * 512],
                                start=(mb == 0),
                                stop=(mb == MB - 1),
                            )
                        emit_norm(pnb, ph, ppo)
                        pend = None
                    if t >= 1:
                        pmb0, pw = TILES[t - 1]
                        for j in range(pw):
                            mb = pmb0 + j
                            # [y_u^T; denom] += [V_h | 1]^T . exp(S^T)
                            nc.tensor.matmul(
                                po[0:65, :],
                                lhsT=v2[:, mb * 130 + 65 * h:mb * 130 + 65 * h + 65],
                                rhs=esL[t - 1][:, j * 512:(j + 1) * 512],
                                start=(mb == 0),
                                stop=(mb == MB - 1),
                            )
                pend = (nb, h, po, esL[-1])
        pnb, ph, ppo, pes = pend
        pmb0, pw = TILES[-1]
        for j in range(pw):
            mb = pmb0 + j
            nc.tensor.matmul(
                ppo[0:65, :],
                lhsT=v2[:, mb * 130 + 65 * ph:mb * 130 + 65 * ph + 65],
                rhs=pes[:, j * 512:(j + 1) * 512],
                start=(mb == 0),
                stop=(mb == MB - 1),
            )
        emit_norm(pnb, ph, ppo)
        emit_proj(NB - 1)

    nc.compile()
    return nc


_NC_CACHE = None
LAST_EXEC_NS = None


def kernel(x, w_qkv, w_proj, b_proj):
    global _NC_CACHE, LAST_EXEC_NS
    x = np.ascontiguousarray(np.asarray(x, dtype=np.float32))
    w_qkv = np.asarray(w_qkv, dtype=np.float32)
    w_proj = np.asarray(w_proj, dtype=np.float32)
    b_proj = np.asarray(b_proj, dtype=np.float32)
    B = x.shape[0]

    if _NC_CACHE is None:
        _NC_CACHE = build_nc()
    nc = _NC_CACHE

    xTs = [np.ascontiguousarray(x[b].T) for b in range(B)]
    in_maps = []
    for c in range(8):
        b, hp = c // 4, c % 4
        qr = w_qkv[2 * hp * 64:2 * hp * 64 + 128]
        kr = w_qkv[512 + 2 * hp * 64:512 + 2 * hp * 64 + 128]
        vr = w_qkv[1024 + 2 * hp * 64:1024 + 2 * hp * 64 + 128]
        wqkvT = np.ascontiguousarray(np.concatenate([qr, kr, vr], 0).T)
        wpT = np.ascontiguousarray(w_proj[:, hp * 128:(hp + 1) * 128].T)
        in_maps.append({"xT": xTs[b], "wqkvT": wqkvT, "wpT": wpT})

    res = run_bass_kernel_spmd(
        nc,
        in_maps,
        core_ids=list(range(8)),
        trace=bool(int(os.environ.get("ATTN_TRACE", "0"))),
    )
    LAST_EXEC_NS = res.exec_time_ns

    out = np.zeros((B, N, DIM), np.float32)
    for b in range(B):
        acc = res.results[4 * b]["out"].copy()
        for c in range(4 * b + 1, 4 * b + 4):
            acc += res.results[c]["out"]
        out[b] = acc.T + b_proj
    return out

